# revision 12
# baseline (speedup 1.0000x reference)
"""Trainium2 Bass kernel for a GNN message-passing layer (8-core SPMD).

Math (reference):
    h   = [x[row], x[col], edge_attr] @ W1 + b1        # [E, 258] @ [258, 128]
    m   = relu(LN(h, g1, be1))
    m   = relu(m @ W2 + b2)
    aggr= segment_sum(m, row, N)
    u   = relu(LN([x, aggr] @ Wu + bu, gu, beu))
    out = x + u

v4 design (vs v3):
  * The Q[col] dma_gather (GpSimd ~8-10ns/idx, 100k idx/core = the v3
    bottleneck at 87% GpSimd occupancy) is GONE: the host streams
    x[col]^T tiles (a pure permutation of the input, like the one-hot
    masks) and the device folds them in with a second accumulating
    matmul against W1b.  No Q table, no AllGathers.
  * LN mean is eliminated algebraically: all of W1/b1 (and Wu/bu) are
    centered on host (W' = W(I - J/D)) so the matmul output is already
    mean-free; LN reduces to z * rsqrt(mean(z^2) + eps).
  * relu(LN(z)) = relu(z)*rstd for rstd>0 and trivial gamma/beta, so
    the normalize+relu is one batched DVE scalar_tensor_tensor with a
    0-stride broadcast of rstd; var comes from a batched scalar-engine
    Square + one segmented vector reduce (no per-tile bn_stats).
  * P[row]+attr pickup stays the v3 one-hot gm matmul trick; scatter
    stays the one-hot oh matmul accumulating per 126-row window.
  * All elementwise work batched over GB=4 tiles ([128, 512] ops) to
    amortize the ~150-300 cycle per-instruction engine overheads.

Edges row-sharded over 8 cores; rows bucketed into 126-node windows
(126 + 2 attr rows = 128 = PE contraction).  Tile order window-major so
one PSUM agg accumulator is live per window.
"""

import math
import os
import numpy as np

D = 128
N_NODES = 50000
N_EDGES = 800000
N_CORES = 8
EPS = 1e-5
P = 128
WN = 126        # nodes per window
CW = 4          # windows per chunk
GB = 4          # tiles per batch group


class Cfg:
    def __init__(self, n_nodes, n_edges, n_cores):
        self.n_nodes = n_nodes
        self.n_edges = n_edges
        self.n_cores = n_cores
        assert n_nodes % n_cores == 0
        self.nloc = n_nodes // n_cores
        self.n_win = math.ceil(self.nloc / WN)
        self.nloc_pad = self.n_win * WN
        self.nt = None          # [n_win] tiles per window
        self.t_total = None
        self.e_pad = None
        self.chunks = None
        self.tiles = None
        self.triv = None


# ---------------------------------------------------------------- host prep

def _ilv128(v):
    """Slot j -> (tile j//128, lane j%128); returns [128, t_total]."""
    return np.ascontiguousarray(v.reshape(-1, P).T)


def plan(cfg):
    """Window-major tile order; chunks of CW windows."""
    nt = cfg.nt
    chunks = []
    tiles = []
    jt = 0
    for c0 in range(0, cfg.n_win, CW):
        wins = list(range(c0, min(c0 + CW, cfg.n_win)))
        jt0 = jt
        for w in wins:
            n = int(nt[w])
            for i in range(n):
                tiles.append(dict(w=w, jt=jt,
                                  is_first=(i == 0), is_last=(i == n - 1)))
                jt += 1
        chunks.append(dict(jt0=jt0, jt1=jt, wins=wins))
    cfg.t_total = jt
    cfg.e_pad = jt * P
    cfg.chunks = chunks
    cfg.tiles = tiles


def preprocess(cfg, x, edge_index, edge_attr):
    rows = np.asarray(edge_index[0], dtype=np.int64)
    cols = np.asarray(edge_index[1], dtype=np.int64)
    attr = np.asarray(edge_attr, dtype=np.float32)

    order = np.argsort(rows, kind="stable")
    rs = rows[order]
    bounds = np.searchsorted(rs, np.arange(cfg.n_cores + 1) * cfg.nloc)

    per_core = []
    counts = np.zeros((cfg.n_cores, cfg.n_win), dtype=np.int64)
    for k in range(cfg.n_cores):
        sel = order[bounds[k]:bounds[k + 1]]
        row_l = (rows[sel] - k * cfg.nloc).astype(np.int32)
        col = cols[sel].astype(np.int32)
        at = attr[sel]
        w = row_l // WN
        o2 = np.argsort(w, kind="stable")
        row_l, col, at, w = row_l[o2], col[o2], at[o2], w[o2]
        cnt = np.bincount(w, minlength=cfg.n_win).astype(np.int64)
        counts[k] = cnt
        per_core.append((row_l, col, at, cnt))

    cnt_max = counts.max(axis=0)
    nt = np.ceil(cnt_max / P).astype(np.int64)
    nt[nt == 0] = 1
    cfg.nt = nt
    plan(cfg)

    import ml_dtypes
    bf16 = ml_dtypes.bfloat16
    xbf = x.astype(bf16)

    core_arrays = []
    for k in range(cfg.n_cores):
        row_l, col, at, cnt = per_core[k]
        qcol = np.zeros(cfg.e_pad, dtype=np.int64)
        sidx = np.full(cfg.e_pad, -1, dtype=np.int32)
        a0 = np.zeros(cfg.e_pad, dtype=np.float32)
        a1 = np.zeros(cfg.e_pad, dtype=np.float32)
        src = 0
        jt = 0
        for w in range(cfg.n_win):
            c = int(cnt[w])
            dst = jt * P
            sl = slice(dst, dst + c)
            seg = slice(src, src + c)
            qcol[sl] = col[seg]
            sidx[sl] = row_l[seg] - w * WN
            a0[sl] = at[seg, 0]
            a1[sl] = at[seg, 1]
            src += c
            jt += int(cfg.nt[w])
        assert src == len(row_l) and jt == cfg.t_total

        sidx_i = _ilv128(sidx)                        # [128(lane), t_total]
        a0i = _ilv128(a0).astype(bf16)
        a1i = _ilv128(a1).astype(bf16)

        gm = np.zeros((P, cfg.t_total, P), dtype=bf16)  # [p, t, lane]
        lane_g, t_g = np.nonzero(sidx_i >= 0)
        gm[sidx_i[lane_g, t_g], t_g, lane_g] = 1
        gm[WN, :, :] = a0i.T
        gm[WN + 1, :, :] = a1i.T

        oh = np.zeros((P, cfg.t_total, WN), dtype=bf16)  # [lane, t, r]
        oh[lane_g, t_g, sidx_i[lane_g, t_g]] = 1

        xcolT = np.ascontiguousarray(xbf[qcol].T)        # [128(k), e_pad]

        core_arrays.append(dict(
            gm=np.ascontiguousarray(gm.reshape(P, cfg.t_total * P)),
            oh=np.ascontiguousarray(oh.reshape(P, cfg.t_total * WN)),
            xcolT=xcolT,
        ))
    return core_arrays


# ---------------------------------------------------------------- device IR

def build(nc, tc, cfg, io):
    import concourse.bass as bass
    from concourse import mybir
    from concourse.masks import make_identity
    from contextlib import ExitStack

    f32 = mybir.dt.float32
    bf16 = mybir.dt.bfloat16
    AF = mybir.ActivationFunctionType
    OP = mybir.AluOpType
    AX = mybir.AxisListType
    triv = cfg.triv

    ctx = ExitStack()
    sing = ctx.enter_context(tc.tile_pool(name="sing", bufs=1))
    work = ctx.enter_context(tc.tile_pool(name="work", bufs=3))
    gat = ctx.enter_context(tc.tile_pool(name="gat", bufs=2))
    stat = ctx.enter_context(tc.tile_pool(name="stat", bufs=3))

    def load_w(name):
        t = sing.tile([P, D], bf16, name=f"{name}_sb")
        nc.sync.dma_start(out=t[:], in_=io[name][:])
        return t

    w1a, w1b, w2, wux, wua = (load_w(n) for n in
                              ("W1a", "W1b", "W2", "Wu_x", "Wu_a"))

    def bcast_row(name, dtype=bf16):
        t = sing.tile([P, D], dtype, name=f"{name}_b")
        src = io[name]
        ap = bass.AP(tensor=src.tensor, offset=src.offset,
                     ap=[[0, P]] + list(src.ap))
        nc.sync.dma_start(out=t[:], in_=ap)
        return t

    b1_b = None if triv["b1"] else bcast_row("b1")
    b2_b = None if triv["b2"] else bcast_row("b2")
    bu_b = None if triv["bu"] else bcast_row("bu", f32)
    g1_b = None if triv["g1"] else bcast_row("g1")
    be1_b = None if triv["be1"] else bcast_row("be1")
    gu_b = None if triv["gu"] else bcast_row("gu", f32)
    beu_b = None if triv["beu"] else bcast_row("beu", f32)

    ident_b = sing.tile([P, P], bf16, name="ident_b")
    make_identity(nc, ident_b[:])

    eps_t = sing.tile([P, 1], f32, name="eps_t")
    nc.vector.memset(eps_t[:], EPS)

    xT = sing.tile([P, cfg.nloc_pad], bf16, name="xT_sb")
    nc.sync.dma_start(out=xT[:], in_=io["xT_local"][:])

    # P table resident in SBUF: rows 0..125 = window nodes, 126/127 = c0/c1
    pc = sing.tile([P, cfg.n_win, D], bf16, name="pc_sb")
    for j, nm in enumerate(("c0", "c1")):
        src = io[nm]
        ap = bass.AP(tensor=src.tensor, offset=src.offset,
                     ap=[[0, 1], [0, cfg.n_win]] + list(src.ap))
        nc.sync.dma_start(out=pc[WN + j:WN + j + 1, :, :], in_=ap)

    # ---------------- phase A: P' = x @ W1a' (+ b1') per window
    with tc.tile_pool(name="psA", bufs=4, space="PSUM") as psA:
        for t in range(cfg.n_win):
            n0 = t * WN
            pp = psA.tile([WN, D], f32, name="pp", tag="mmA", bufs=4)
            nc.tensor.matmul(pp[:], lhsT=xT[:, n0:n0 + WN], rhs=w1a[:],
                             start=True, stop=True)
            if b1_b is None:
                nc.vector.tensor_copy(out=pc[0:WN, t, :], in_=pp[:])
            else:
                nc.vector.tensor_add(out=pc[0:WN, t, :], in0=pp[:],
                                     in1=b1_b[0:WN, :])

    # ---------------- phase B: edge pipeline
    out_dram = io["out"]
    t_ch = max(ch["jt1"] - ch["jt0"] for ch in cfg.chunks)

    psB = ctx.enter_context(tc.tile_pool(name="psB", bufs=1, space="PSUM"))
    cur_agg = [None]
    gctr = [0]

    for ch in cfg.chunks:
        jt0, jt1 = ch["jt0"], ch["jt1"]
        ct = jt1 - jt0

        gm = gat.tile([P, t_ch * P], bf16, name="gm", tag="gm")
        nc.sync.dma_start(out=gm[:, 0:ct * P],
                          in_=io["gm"][:, jt0 * P:jt1 * P])
        xc = gat.tile([P, t_ch * P], bf16, name="xc", tag="xc")
        nc.sync.dma_start(out=xc[:, 0:ct * P],
                          in_=io["xcolT"][:, jt0 * P:jt1 * P])
        oh = gat.tile([P, t_ch * WN], bf16, name="oh", tag="oh")
        nc.sync.dma_start(out=oh[:, 0:ct * WN],
                          in_=io["oh"][:, jt0 * WN:jt1 * WN])

        for g0 in range(0, ct, GB):
            gsz = min(GB, ct - g0)
            tl = [cfg.tiles[jt0 + g0 + i] for i in range(gsz)]

            # z_c = P'[row] + attr@C' + x[col]@W1b'   (mean-free by W centering)
            zp = psB.tile([P, GB, D], f32, name="zp", tag="zp", bufs=2)
            for i in range(gsz):
                jl = g0 + i
                nc.tensor.matmul(zp[:, i, :],
                                 lhsT=gm[:, jl * P:(jl + 1) * P],
                                 rhs=pc[:, tl[i]["w"], :],
                                 start=True, stop=False)
                nc.tensor.matmul(zp[:, i, :],
                                 lhsT=xc[:, jl * P:(jl + 1) * P],
                                 rhs=w1b[:],
                                 start=False, stop=True)

            # rstd = rsqrt(mean(z^2) + eps)
            zsq = work.tile([P, GB, D], bf16, name="zsq", tag="zsq")
            nc.scalar.activation(out=zsq[:, 0:gsz, :], in_=zp[:, 0:gsz, :],
                                 func=AF.Square)
            ssq = stat.tile([P, GB], f32, name="ssq", tag="ssq")
            nc.vector.tensor_reduce(out=ssq[:, 0:gsz], in_=zsq[:, 0:gsz, :],
                                    axis=AX.X, op=OP.add)
            sdv = stat.tile([P, GB], f32, name="sdv", tag="sdv")
            nc.scalar.activation(out=sdv[:, 0:gsz], in_=ssq[:, 0:gsz],
                                 func=AF.Sqrt, bias=eps_t[:], scale=1.0 / D)
            rstd = stat.tile([P, GB], f32, name="rstd", tag="rstd")
            nc.vector.reciprocal(out=rstd[:, 0:gsz], in_=sdv[:, 0:gsz])

            # m1 = relu(z)*rstd  (== relu(LN(z)) for trivial gamma/beta)
            m1g = work.tile([P, GB, D], bf16, name="m1g", tag="m1g")
            if triv["g1"] and triv["be1"]:
                rb = rstd[:, 0:gsz].unsqueeze(2).to_broadcast([P, gsz, D])
                nc.vector.scalar_tensor_tensor(
                    out=m1g[:, 0:gsz, :], in0=zp[:, 0:gsz, :], scalar=0.0,
                    op0=OP.max, in1=rb, op1=OP.mult)
            else:
                rb = rstd[:, 0:gsz].unsqueeze(2).to_broadcast([P, gsz, D])
                nc.vector.scalar_tensor_tensor(
                    out=m1g[:, 0:gsz, :], in0=zp[:, 0:gsz, :], scalar=1.0,
                    op0=OP.mult, in1=rb, op1=OP.mult)
                for i in range(gsz):
                    nc.vector.tensor_mul(out=m1g[:, i, :], in0=m1g[:, i, :],
                                         in1=g1_b[:])
                    nc.vector.tensor_add(out=m1g[:, i, :], in0=m1g[:, i, :],
                                         in1=be1_b[:])
                nc.vector.tensor_scalar_max(out=m1g[:, 0:gsz, :],
                                            in0=m1g[:, 0:gsz, :], scalar1=0.0)

            trp = psB.tile([P, GB, D], bf16, name="trp", tag="trp", bufs=2)
            for i in range(gsz):
                nc.tensor.transpose(trp[:, i, :], in_=m1g[:, i, :],
                                    identity=ident_b[:])
            # balance the PSUM->SBUF copy: every 3rd group on the scalar
            # engine (its queue has ~70us of slack vs the vector wall)
            m1t = work.tile([P, GB, D], bf16, name="m1t", tag="m1t")
            if gctr[0] % 3 == 2:
                nc.scalar.copy(out=m1t[:, 0:gsz, :], in_=trp[:, 0:gsz, :])
            else:
                nc.vector.tensor_copy(out=m1t[:, 0:gsz, :],
                                      in_=trp[:, 0:gsz, :])
            gctr[0] += 1

            m2p = psB.tile([P, GB, D], f32, name="m2p", tag="m2p", bufs=1)
            for i in range(gsz):
                nc.tensor.matmul(m2p[:, i, :], lhsT=m1t[:, i, :], rhs=w2[:],
                                 start=True, stop=True)
            m2 = work.tile([P, GB, D], bf16, name="m2", tag="m2")
            if b2_b is None:
                nc.scalar.activation(out=m2[:, 0:gsz, :], in_=m2p[:, 0:gsz, :],
                                     func=AF.Relu)
            else:
                nc.vector.tensor_add(out=m2[:, 0:gsz, :], in0=m2p[:, 0:gsz, :],
                                     in1=b2_b[:])
                nc.scalar.activation(out=m2[:, 0:gsz, :], in_=m2[:, 0:gsz, :],
                                     func=AF.Relu)

            for i in range(gsz):
                t = tl[i]
                jl = g0 + i
                if t["is_first"]:
                    cur_agg[0] = psB.tile([P, WN], f32, name="agg",
                                          tag="agg", bufs=2)
                nc.tensor.matmul(cur_agg[0][:], lhsT=m2[:, i, :],
                                 rhs=oh[:, jl * WN:(jl + 1) * WN],
                                 start=t["is_first"], stop=t["is_last"])
                if t["is_last"]:
                    _update_window(nc, cfg, io, t["w"], cur_agg[0][:],
                                   xT, wux, wua, bu_b, gu_b, beu_b, eps_t,
                                   work, stat, psB, out_dram, triv, mybir)

    ctx.close()


def _update_window(nc, cfg, io, w, agg, xT, wux, wua, bu_b, gu_b, beu_b,
                   eps_t, work, stat, psB, out_dram, triv, mybir):
    f32 = mybir.dt.float32
    bf16 = mybir.dt.bfloat16
    AF = mybir.ActivationFunctionType
    OP = mybir.AluOpType
    AX = mybir.AxisListType
    n0 = w * WN

    aggs = work.tile([P, WN], bf16, name="aggs", tag="aggs")
    nc.scalar.copy(out=aggs[:], in_=agg)

    up = psB.tile([WN, D], f32, name="up", tag="up", bufs=1)
    nc.tensor.matmul(up[:], lhsT=aggs[:], rhs=wua[:], start=True, stop=False)
    nc.tensor.matmul(up[:], lhsT=xT[:, n0:n0 + WN], rhs=wux[:],
                     start=False, stop=True)
    if bu_b is not None:
        nc.vector.tensor_add(out=up[:], in0=up[:], in1=bu_b[0:WN, :])

    usq = work.tile([WN, D], bf16, name="usq", tag="usq")
    nc.scalar.activation(out=usq[:], in_=up[:], func=AF.Square)
    uss = stat.tile([WN, 1], f32, name="uss", tag="uss")
    nc.vector.tensor_reduce(out=uss[:], in_=usq[:], axis=AX.X, op=OP.add)
    usd = stat.tile([WN, 1], f32, name="usd", tag="usd")
    nc.scalar.activation(out=usd[:], in_=uss[:], func=AF.Sqrt,
                         bias=eps_t[0:WN, :], scale=1.0 / D)
    rsu = stat.tile([WN, 1], f32, name="rsu", tag="rsu")
    nc.vector.reciprocal(out=rsu[:], in_=usd[:])

    u_sb = work.tile([WN, D], f32, name="u_sb", tag="u_sb")
    if triv["gu"] and triv["beu"]:
        nc.vector.scalar_tensor_tensor(
            out=u_sb[:], in0=up[:], scalar=0.0, op0=OP.max,
            in1=rsu[:].to_broadcast([WN, D]), op1=OP.mult)
    else:
        nc.vector.scalar_tensor_tensor(
            out=u_sb[:], in0=up[:], scalar=1.0, op0=OP.mult,
            in1=rsu[:].to_broadcast([WN, D]), op1=OP.mult)
        nc.vector.tensor_mul(out=u_sb[:], in0=u_sb[:], in1=gu_b[0:WN, :])
        nc.vector.tensor_add(out=u_sb[:], in0=u_sb[:], in1=beu_b[0:WN, :])
        nc.vector.tensor_scalar_max(out=u_sb[:], in0=u_sb[:], scalar1=0.0)

    xw = work.tile([WN, D], f32, name="xw", tag="xw")
    nc.sync.dma_start(out=xw[:], in_=io["x_local"][n0:n0 + WN, :])
    o_sb = work.tile([WN, D], f32, name="o_sb", tag="o_sb")
    nc.vector.tensor_add(out=o_sb[:], in0=u_sb[:], in1=xw[:])
    nc.sync.dma_start(out=out_dram[n0:n0 + WN, :], in_=o_sb[:])


def make_program(cfg):
    import concourse.bacc as bacc
    import concourse.tile as tile
    from concourse import mybir

    f32 = mybir.dt.float32
    bf16 = mybir.dt.bfloat16

    nc = bacc.Bacc("TRN2", target_bir_lowering=False, debug=False,
                   num_devices=cfg.n_cores)
    io = {}

    def din(name, shape, dtype=f32):
        io[name] = nc.dram_tensor(name, list(shape), dtype,
                                  kind="ExternalInput").ap()

    din("xT_local", [P, cfg.nloc_pad], bf16)
    din("x_local", [cfg.nloc_pad, D])
    din("gm", [P, cfg.t_total * P], bf16)
    din("oh", [P, cfg.t_total * WN], bf16)
    din("xcolT", [P, cfg.t_total * P], bf16)
    for nm in ("W1a", "W1b", "W2", "Wu_x", "Wu_a"):
        din(nm, [P, D], bf16)
    for nm in ("c0", "c1", "b1", "b2", "g1", "be1"):
        din(nm, [D], bf16)
    for nm in ("bu", "gu", "beu"):
        din(nm, [D])
    io["out"] = nc.dram_tensor("out", [cfg.nloc_pad, D], f32,
                               kind="ExternalOutput").ap()

    with tile.TileContext(nc) as tc:
        build(nc, tc, cfg, io)
    nc.compile()
    return nc


# ---------------------------------------------------------------- entry

def _is_const(v, val):
    return bool(np.allclose(np.asarray(v), val))


def _center(w):
    """Right-multiply by (I - J/D): subtract each row's mean."""
    return w - w.mean(axis=1, keepdims=True)


def kernel(x, edge_index, edge_attr, W1, b1, g1, be1, W2, b2, Wu, bu, gu, beu,
           cfg=None, run=True):
    import ml_dtypes
    bf16 = ml_dtypes.bfloat16

    x = np.ascontiguousarray(np.asarray(x, dtype=np.float32))
    edge_index = np.asarray(edge_index)
    edge_attr = np.asarray(edge_attr, dtype=np.float32)
    W1 = np.asarray(W1, dtype=np.float32)
    W2 = np.ascontiguousarray(np.asarray(W2, dtype=np.float32))
    Wu = np.asarray(Wu, dtype=np.float32)
    b1 = np.asarray(b1, np.float32)
    bu = np.asarray(bu, np.float32)

    if cfg is None:
        cfg = Cfg(N_NODES, N_EDGES, N_CORES)
    cfg.triv = dict(
        b1=_is_const(b1, 0), b2=_is_const(b2, 0), bu=_is_const(bu, 0),
        g1=_is_const(g1, 1), be1=_is_const(be1, 0),
        gu=_is_const(gu, 1), beu=_is_const(beu, 0),
    )

    core_arrays = preprocess(cfg, x, edge_index, edge_attr)

    # Center the pre-LN linear maps so matmul outputs are mean-free.
    W1c = _center(W1)
    b1c = b1 - b1.mean()
    Wuc = _center(Wu)
    buc = bu - bu.mean()

    weights = dict(
        W1a=np.ascontiguousarray(W1c[0:D]).astype(bf16),
        W1b=np.ascontiguousarray(W1c[D:2 * D]).astype(bf16),
        W2=W2.astype(bf16),
        Wu_x=np.ascontiguousarray(Wuc[0:D]).astype(bf16),
        Wu_a=np.ascontiguousarray(Wuc[D:2 * D]).astype(bf16),
        c0=np.ascontiguousarray(W1c[2 * D]).astype(bf16),
        c1=np.ascontiguousarray(W1c[2 * D + 1]).astype(bf16),
        b1=b1c.astype(bf16),
        b2=np.asarray(b2, np.float32).astype(bf16),
        g1=np.asarray(g1, np.float32).astype(bf16),
        be1=np.asarray(be1, np.float32).astype(bf16),
        bu=np.ascontiguousarray(buc),
        gu=np.ascontiguousarray(np.asarray(gu, np.float32)),
        beu=np.ascontiguousarray(np.asarray(beu, np.float32)),
    )

    nc = make_program(cfg)

    in_maps = []
    for k in range(cfg.n_cores):
        xl = np.zeros((cfg.nloc_pad, D), dtype=np.float32)
        xl[:cfg.nloc] = x[k * cfg.nloc:(k + 1) * cfg.nloc]
        m = dict(core_arrays[k])
        m["x_local"] = xl
        m["xT_local"] = np.ascontiguousarray(xl.T).astype(bf16)
        m.update(weights)
        in_maps.append(m)

    if not run:
        return nc, in_maps, cfg

    from concourse import bass_utils
    res = bass_utils.run_bass_kernel_spmd(
        nc, in_maps, core_ids=list(range(cfg.n_cores)),
        trace=bool(int(os.environ.get("KERNEL_TRACE", "0"))),
    )
    kernel.last_results = res
    outs = [r["out"][:cfg.nloc] for r in res.results]
    return np.concatenate(outs, axis=0)


kernel.last_results = None


# revision 15
# speedup vs baseline: 1.0322x; 1.0322x over previous
"""Trainium2 Bass kernel for a GNN message-passing layer (8-core SPMD).

Math (reference):
    h   = [x[row], x[col], edge_attr] @ W1 + b1        # [E, 258] @ [258, 128]
    m   = relu(LN(h, g1, be1))
    m   = relu(m @ W2 + b2)
    aggr= segment_sum(m, row, N)
    u   = relu(LN([x, aggr] @ Wu + bu, gu, beu))
    out = x + u

v4 design (vs v3):
  * The Q[col] dma_gather (GpSimd ~8-10ns/idx, 100k idx/core = the v3
    bottleneck at 87% GpSimd occupancy) is GONE: the host streams
    x[col]^T tiles (a pure permutation of the input, like the one-hot
    masks) and the device folds them in with a second accumulating
    matmul against W1b.  No Q table, no AllGathers.
  * LN mean is eliminated algebraically: all of W1/b1 (and Wu/bu) are
    centered on host (W' = W(I - J/D)) so the matmul output is already
    mean-free; LN reduces to z * rsqrt(mean(z^2) + eps).
  * relu(LN(z)) = relu(z)*rstd for rstd>0 and trivial gamma/beta, so
    the normalize+relu is one batched DVE scalar_tensor_tensor with a
    0-stride broadcast of rstd; var comes from a batched scalar-engine
    Square + one segmented vector reduce (no per-tile bn_stats).
  * P[row]+attr pickup stays the v3 one-hot gm matmul trick; scatter
    stays the one-hot oh matmul accumulating per 126-row window.
  * All elementwise work batched over GB=4 tiles ([128, 512] ops) to
    amortize the ~150-300 cycle per-instruction engine overheads.

Edges row-sharded over 8 cores; rows bucketed into 126-node windows
(126 + 2 attr rows = 128 = PE contraction).  Tile order window-major so
one PSUM agg accumulator is live per window.
"""

import math
import os
import numpy as np

D = 128
N_NODES = 50000
N_EDGES = 800000
N_CORES = 8
EPS = 1e-5
P = 128
WN = 126        # nodes per window
CW = 4          # windows per chunk
GB = 4          # tiles per batch group


class Cfg:
    def __init__(self, n_nodes, n_edges, n_cores):
        self.n_nodes = n_nodes
        self.n_edges = n_edges
        self.n_cores = n_cores
        assert n_nodes % n_cores == 0
        self.nloc = n_nodes // n_cores
        self.n_win = math.ceil(self.nloc / WN)
        self.nloc_pad = self.n_win * WN
        self.nt = None          # [n_win] tiles per window
        self.t_total = None
        self.e_pad = None
        self.chunks = None
        self.tiles = None
        self.triv = None


# ---------------------------------------------------------------- host prep

def _ilv128(v):
    """Slot j -> (tile j//128, lane j%128); returns [128, t_total]."""
    return np.ascontiguousarray(v.reshape(-1, P).T)


def plan(cfg):
    """Window-major tile order; chunks of CW windows."""
    nt = cfg.nt
    chunks = []
    tiles = []
    jt = 0
    for c0 in range(0, cfg.n_win, CW):
        wins = list(range(c0, min(c0 + CW, cfg.n_win)))
        jt0 = jt
        for w in wins:
            n = int(nt[w])
            for i in range(n):
                tiles.append(dict(w=w, jt=jt,
                                  is_first=(i == 0), is_last=(i == n - 1)))
                jt += 1
        chunks.append(dict(jt0=jt0, jt1=jt, wins=wins))
    cfg.t_total = jt
    cfg.e_pad = jt * P
    cfg.chunks = chunks
    cfg.tiles = tiles


def preprocess(cfg, x, edge_index, edge_attr):
    rows = np.asarray(edge_index[0], dtype=np.int64)
    cols = np.asarray(edge_index[1], dtype=np.int64)
    attr = np.asarray(edge_attr, dtype=np.float32)

    order = np.argsort(rows, kind="stable")
    rs = rows[order]
    bounds = np.searchsorted(rs, np.arange(cfg.n_cores + 1) * cfg.nloc)

    per_core = []
    counts = np.zeros((cfg.n_cores, cfg.n_win), dtype=np.int64)
    for k in range(cfg.n_cores):
        sel = order[bounds[k]:bounds[k + 1]]
        row_l = (rows[sel] - k * cfg.nloc).astype(np.int32)
        col = cols[sel].astype(np.int32)
        at = attr[sel]
        w = row_l // WN
        o2 = np.argsort(w, kind="stable")
        row_l, col, at, w = row_l[o2], col[o2], at[o2], w[o2]
        cnt = np.bincount(w, minlength=cfg.n_win).astype(np.int64)
        counts[k] = cnt
        per_core.append((row_l, col, at, cnt))

    cnt_max = counts.max(axis=0)
    nt = np.ceil(cnt_max / P).astype(np.int64)
    nt[nt == 0] = 1
    cfg.nt = nt
    plan(cfg)

    import ml_dtypes
    bf16 = ml_dtypes.bfloat16
    xbf = x.astype(bf16)

    core_arrays = []
    for k in range(cfg.n_cores):
        row_l, col, at, cnt = per_core[k]
        qcol = np.zeros(cfg.e_pad, dtype=np.int64)
        sidx = np.full(cfg.e_pad, -1, dtype=np.int32)
        a0 = np.zeros(cfg.e_pad, dtype=np.float32)
        a1 = np.zeros(cfg.e_pad, dtype=np.float32)
        src = 0
        jt = 0
        for w in range(cfg.n_win):
            c = int(cnt[w])
            dst = jt * P
            sl = slice(dst, dst + c)
            seg = slice(src, src + c)
            qcol[sl] = col[seg]
            sidx[sl] = row_l[seg] - w * WN
            a0[sl] = at[seg, 0]
            a1[sl] = at[seg, 1]
            src += c
            jt += int(cfg.nt[w])
        assert src == len(row_l) and jt == cfg.t_total

        sidx_i = _ilv128(sidx)                        # [128(lane), t_total]
        a0i = _ilv128(a0).astype(bf16)
        a1i = _ilv128(a1).astype(bf16)

        gm = np.zeros((P, cfg.t_total, P), dtype=bf16)  # [p, t, lane]
        lane_g, t_g = np.nonzero(sidx_i >= 0)
        gm[sidx_i[lane_g, t_g], t_g, lane_g] = 1
        gm[WN, :, :] = a0i.T
        gm[WN + 1, :, :] = a1i.T

        oh = np.zeros((P, cfg.t_total, WN), dtype=bf16)  # [lane, t, r]
        oh[lane_g, t_g, sidx_i[lane_g, t_g]] = 1

        xcolT = np.ascontiguousarray(xbf[qcol].T)        # [128(k), e_pad]

        core_arrays.append(dict(
            gm=np.ascontiguousarray(gm.reshape(P, cfg.t_total * P)),
            oh=np.ascontiguousarray(oh.reshape(P, cfg.t_total * WN)),
            xcolT=xcolT,
        ))
    return core_arrays


# ---------------------------------------------------------------- device IR

def build(nc, tc, cfg, io):
    import concourse.bass as bass
    from concourse import mybir
    from concourse.masks import make_identity
    from contextlib import ExitStack

    f32 = mybir.dt.float32
    bf16 = mybir.dt.bfloat16
    AF = mybir.ActivationFunctionType
    OP = mybir.AluOpType
    AX = mybir.AxisListType
    triv = cfg.triv

    ctx = ExitStack()
    sing = ctx.enter_context(tc.tile_pool(name="sing", bufs=1))
    work = ctx.enter_context(tc.tile_pool(name="work", bufs=3))
    gat = ctx.enter_context(tc.tile_pool(name="gat", bufs=2))
    stat = ctx.enter_context(tc.tile_pool(name="stat", bufs=3))

    def load_w(name):
        t = sing.tile([P, D], bf16, name=f"{name}_sb")
        nc.sync.dma_start(out=t[:], in_=io[name][:])
        return t

    w1a, w1b, w2, wux, wua = (load_w(n) for n in
                              ("W1a", "W1b", "W2", "Wu_x", "Wu_a"))

    def bcast_row(name, dtype=bf16):
        t = sing.tile([P, D], dtype, name=f"{name}_b")
        src = io[name]
        ap = bass.AP(tensor=src.tensor, offset=src.offset,
                     ap=[[0, P]] + list(src.ap))
        nc.sync.dma_start(out=t[:], in_=ap)
        return t

    b1_b = None if triv["b1"] else bcast_row("b1")
    b2_b = None if triv["b2"] else bcast_row("b2")
    bu_b = None if triv["bu"] else bcast_row("bu", f32)
    g1_b = None if triv["g1"] else bcast_row("g1")
    be1_b = None if triv["be1"] else bcast_row("be1")
    gu_b = None if triv["gu"] else bcast_row("gu", f32)
    beu_b = None if triv["beu"] else bcast_row("beu", f32)

    ident_b = sing.tile([P, P], bf16, name="ident_b")
    make_identity(nc, ident_b[:])

    eps_t = sing.tile([P, 1], f32, name="eps_t")
    nc.vector.memset(eps_t[:], EPS)

    xT = sing.tile([P, cfg.nloc_pad], bf16, name="xT_sb")
    nc.sync.dma_start(out=xT[:], in_=io["xT_local"][:])

    # P table resident in SBUF: rows 0..125 = window nodes, 126/127 = c0/c1
    pc = sing.tile([P, cfg.n_win, D], bf16, name="pc_sb")
    for j, nm in enumerate(("c0", "c1")):
        src = io[nm]
        ap = bass.AP(tensor=src.tensor, offset=src.offset,
                     ap=[[0, 1], [0, cfg.n_win]] + list(src.ap))
        nc.sync.dma_start(out=pc[WN + j:WN + j + 1, :, :], in_=ap)

    # ---------------- phase A: P' = x @ W1a' (+ b1') per window
    with tc.tile_pool(name="psA", bufs=4, space="PSUM") as psA:
        for t in range(cfg.n_win):
            n0 = t * WN
            pp = psA.tile([WN, D], f32, name="pp", tag="mmA", bufs=4)
            nc.tensor.matmul(pp[:], lhsT=xT[:, n0:n0 + WN], rhs=w1a[:],
                             start=True, stop=True)
            if b1_b is None:
                nc.vector.tensor_copy(out=pc[0:WN, t, :], in_=pp[:])
            else:
                nc.vector.tensor_add(out=pc[0:WN, t, :], in0=pp[:],
                                     in1=b1_b[0:WN, :])

    # ---------------- phase B: edge pipeline
    out_dram = io["out"]
    t_ch = max(ch["jt1"] - ch["jt0"] for ch in cfg.chunks)

    psB = ctx.enter_context(tc.tile_pool(name="psB", bufs=1, space="PSUM"))
    cur_agg = [None]
    pend = [None]

    def _finish_a(c):
        # V m1t copy + PE m2 matmuls: independent work slotted between this
        # group's reduce and reciprocal to hide the scalar Sqrt latency.
        gsz = c["gsz"]
        m1t = work.tile([P, GB, D], bf16, name="m1t", tag="m1t")
        nc.vector.tensor_copy(out=m1t[:, 0:gsz, :], in_=c["trp"][:, 0:gsz, :])
        m2p = psB.tile([P, GB, D], f32, name="m2p", tag="m2p", bufs=1)
        for i in range(gsz):
            nc.tensor.matmul(m2p[:, i, :], lhsT=m1t[:, i, :], rhs=w2[:],
                             start=True, stop=True)
        c["m2p"] = m2p

    def _finish_b(c):
        gsz, g0, tl, oh, m2p = c["gsz"], c["g0"], c["tl"], c["oh"], c["m2p"]
        m2 = work.tile([P, GB, D], bf16, name="m2", tag="m2")
        if b2_b is None:
            nc.scalar.activation(out=m2[:, 0:gsz, :], in_=m2p[:, 0:gsz, :],
                                 func=AF.Relu)
        else:
            nc.vector.tensor_add(out=m2[:, 0:gsz, :], in0=m2p[:, 0:gsz, :],
                                 in1=b2_b[:])
            nc.scalar.activation(out=m2[:, 0:gsz, :], in_=m2[:, 0:gsz, :],
                                 func=AF.Relu)
        for i in range(gsz):
            t = tl[i]
            jl = g0 + i
            if t["is_first"]:
                cur_agg[0] = psB.tile([P, WN], f32, name="agg",
                                      tag="agg", bufs=2)
            nc.tensor.matmul(cur_agg[0][:], lhsT=m2[:, i, :],
                             rhs=oh[:, jl * WN:(jl + 1) * WN],
                             start=t["is_first"], stop=t["is_last"])
            if t["is_last"]:
                _update_window(nc, cfg, io, t["w"], cur_agg[0][:],
                               xT, wux, wua, bu_b, gu_b, beu_b, eps_t,
                               work, stat, psB, out_dram, triv, mybir)

    for ch in cfg.chunks:
        jt0, jt1 = ch["jt0"], ch["jt1"]
        ct = jt1 - jt0

        gm = gat.tile([P, t_ch * P], bf16, name="gm", tag="gm")
        nc.sync.dma_start(out=gm[:, 0:ct * P],
                          in_=io["gm"][:, jt0 * P:jt1 * P])
        xc = gat.tile([P, t_ch * P], bf16, name="xc", tag="xc")
        nc.sync.dma_start(out=xc[:, 0:ct * P],
                          in_=io["xcolT"][:, jt0 * P:jt1 * P])
        oh = gat.tile([P, t_ch * WN], bf16, name="oh", tag="oh")
        nc.sync.dma_start(out=oh[:, 0:ct * WN],
                          in_=io["oh"][:, jt0 * WN:jt1 * WN])

        for g0 in range(0, ct, GB):
            gsz = min(GB, ct - g0)
            tl = [cfg.tiles[jt0 + g0 + i] for i in range(gsz)]

            # z_c = P'[row] + attr@C' + x[col]@W1b'   (mean-free by W centering)
            zp = psB.tile([P, GB, D], f32, name="zp", tag="zp", bufs=2)
            for i in range(gsz):
                jl = g0 + i
                nc.tensor.matmul(zp[:, i, :],
                                 lhsT=gm[:, jl * P:(jl + 1) * P],
                                 rhs=pc[:, tl[i]["w"], :],
                                 start=True, stop=False)
                nc.tensor.matmul(zp[:, i, :],
                                 lhsT=xc[:, jl * P:(jl + 1) * P],
                                 rhs=w1b[:],
                                 start=False, stop=True)

            # rstd = rsqrt(mean(z^2) + eps)
            zsq = work.tile([P, GB, D], bf16, name="zsq", tag="zsq")
            nc.scalar.activation(out=zsq[:, 0:gsz, :], in_=zp[:, 0:gsz, :],
                                 func=AF.Square)
            ssq = stat.tile([P, GB], f32, name="ssq", tag="ssq")
            nc.vector.tensor_reduce(out=ssq[:, 0:gsz], in_=zsq[:, 0:gsz, :],
                                    axis=AX.X, op=OP.add)

            # Deferred previous group, part A (V m1t + PE m2-mms): slotted
            # here so V has independent work while S runs this group's Sqrt.
            if pend[0] is not None:
                _finish_a(pend[0])

            sdv = stat.tile([P, GB], f32, name="sdv", tag="sdv")
            nc.scalar.activation(out=sdv[:, 0:gsz], in_=ssq[:, 0:gsz],
                                 func=AF.Sqrt, bias=eps_t[:], scale=1.0 / D)
            rstd = stat.tile([P, GB], f32, name="rstd", tag="rstd")
            nc.vector.reciprocal(out=rstd[:, 0:gsz], in_=sdv[:, 0:gsz])

            # m1 = relu(z)*rstd  (== relu(LN(z)) for trivial gamma/beta)
            m1g = work.tile([P, GB, D], bf16, name="m1g", tag="m1g")
            if triv["g1"] and triv["be1"]:
                rb = rstd[:, 0:gsz].unsqueeze(2).to_broadcast([P, gsz, D])
                nc.vector.scalar_tensor_tensor(
                    out=m1g[:, 0:gsz, :], in0=zp[:, 0:gsz, :], scalar=0.0,
                    op0=OP.max, in1=rb, op1=OP.mult)
            else:
                rb = rstd[:, 0:gsz].unsqueeze(2).to_broadcast([P, gsz, D])
                nc.vector.scalar_tensor_tensor(
                    out=m1g[:, 0:gsz, :], in0=zp[:, 0:gsz, :], scalar=1.0,
                    op0=OP.mult, in1=rb, op1=OP.mult)
                for i in range(gsz):
                    nc.vector.tensor_mul(out=m1g[:, i, :], in0=m1g[:, i, :],
                                         in1=g1_b[:])
                    nc.vector.tensor_add(out=m1g[:, i, :], in0=m1g[:, i, :],
                                         in1=be1_b[:])
                nc.vector.tensor_scalar_max(out=m1g[:, 0:gsz, :],
                                            in0=m1g[:, 0:gsz, :], scalar1=0.0)

            trp = psB.tile([P, GB, D], bf16, name="trp", tag="trp", bufs=2)
            for i in range(gsz):
                nc.tensor.transpose(trp[:, i, :], in_=m1g[:, i, :],
                                    identity=ident_b[:])
            # Deferred previous group, part B (S m2relu + PE scatters):
            # after this group's transposes so S's queue stays sq->sqrt->relu.
            if pend[0] is not None:
                _finish_b(pend[0])
            pend[0] = dict(trp=trp, gsz=gsz, g0=g0, tl=tl, oh=oh)

    if pend[0] is not None:
        _finish_a(pend[0])
        _finish_b(pend[0])
        pend[0] = None

    ctx.close()


def _update_window(nc, cfg, io, w, agg, xT, wux, wua, bu_b, gu_b, beu_b,
                   eps_t, work, stat, psB, out_dram, triv, mybir):
    f32 = mybir.dt.float32
    bf16 = mybir.dt.bfloat16
    AF = mybir.ActivationFunctionType
    OP = mybir.AluOpType
    AX = mybir.AxisListType
    n0 = w * WN

    aggs = work.tile([P, WN], bf16, name="aggs", tag="aggs")
    nc.scalar.copy(out=aggs[:], in_=agg)

    up = psB.tile([WN, D], f32, name="up", tag="up", bufs=1)
    nc.tensor.matmul(up[:], lhsT=aggs[:], rhs=wua[:], start=True, stop=False)
    nc.tensor.matmul(up[:], lhsT=xT[:, n0:n0 + WN], rhs=wux[:],
                     start=False, stop=True)
    if bu_b is not None:
        nc.vector.tensor_add(out=up[:], in0=up[:], in1=bu_b[0:WN, :])

    usq = work.tile([WN, D], bf16, name="usq", tag="usq")
    nc.scalar.activation(out=usq[:], in_=up[:], func=AF.Square)
    uss = stat.tile([WN, 1], f32, name="uss", tag="uss")
    nc.vector.tensor_reduce(out=uss[:], in_=usq[:], axis=AX.X, op=OP.add)
    usd = stat.tile([WN, 1], f32, name="usd", tag="usd")
    nc.scalar.activation(out=usd[:], in_=uss[:], func=AF.Sqrt,
                         bias=eps_t[0:WN, :], scale=1.0 / D)
    rsu = stat.tile([WN, 1], f32, name="rsu", tag="rsu")
    nc.vector.reciprocal(out=rsu[:], in_=usd[:])

    u_sb = work.tile([WN, D], f32, name="u_sb", tag="u_sb")
    if triv["gu"] and triv["beu"]:
        nc.vector.scalar_tensor_tensor(
            out=u_sb[:], in0=up[:], scalar=0.0, op0=OP.max,
            in1=rsu[:].to_broadcast([WN, D]), op1=OP.mult)
    else:
        nc.vector.scalar_tensor_tensor(
            out=u_sb[:], in0=up[:], scalar=1.0, op0=OP.mult,
            in1=rsu[:].to_broadcast([WN, D]), op1=OP.mult)
        nc.vector.tensor_mul(out=u_sb[:], in0=u_sb[:], in1=gu_b[0:WN, :])
        nc.vector.tensor_add(out=u_sb[:], in0=u_sb[:], in1=beu_b[0:WN, :])
        nc.vector.tensor_scalar_max(out=u_sb[:], in0=u_sb[:], scalar1=0.0)

    xw = work.tile([WN, D], f32, name="xw", tag="xw")
    nc.sync.dma_start(out=xw[:], in_=io["x_local"][n0:n0 + WN, :])
    o_sb = work.tile([WN, D], f32, name="o_sb", tag="o_sb")
    nc.vector.tensor_add(out=o_sb[:], in0=u_sb[:], in1=xw[:])
    nc.sync.dma_start(out=out_dram[n0:n0 + WN, :], in_=o_sb[:])


def make_program(cfg):
    import concourse.bacc as bacc
    import concourse.tile as tile
    from concourse import mybir

    f32 = mybir.dt.float32
    bf16 = mybir.dt.bfloat16

    nc = bacc.Bacc("TRN2", target_bir_lowering=False, debug=False,
                   num_devices=cfg.n_cores)
    io = {}

    def din(name, shape, dtype=f32):
        io[name] = nc.dram_tensor(name, list(shape), dtype,
                                  kind="ExternalInput").ap()

    din("xT_local", [P, cfg.nloc_pad], bf16)
    din("x_local", [cfg.nloc_pad, D])
    din("gm", [P, cfg.t_total * P], bf16)
    din("oh", [P, cfg.t_total * WN], bf16)
    din("xcolT", [P, cfg.t_total * P], bf16)
    for nm in ("W1a", "W1b", "W2", "Wu_x", "Wu_a"):
        din(nm, [P, D], bf16)
    for nm in ("c0", "c1", "b1", "b2", "g1", "be1"):
        din(nm, [D], bf16)
    for nm in ("bu", "gu", "beu"):
        din(nm, [D])
    io["out"] = nc.dram_tensor("out", [cfg.nloc_pad, D], f32,
                               kind="ExternalOutput").ap()

    with tile.TileContext(nc) as tc:
        build(nc, tc, cfg, io)
    nc.compile()
    return nc


# ---------------------------------------------------------------- entry

def _is_const(v, val):
    return bool(np.allclose(np.asarray(v), val))


def _center(w):
    """Right-multiply by (I - J/D): subtract each row's mean."""
    return w - w.mean(axis=1, keepdims=True)


def kernel(x, edge_index, edge_attr, W1, b1, g1, be1, W2, b2, Wu, bu, gu, beu,
           cfg=None, run=True):
    import ml_dtypes
    bf16 = ml_dtypes.bfloat16

    x = np.ascontiguousarray(np.asarray(x, dtype=np.float32))
    edge_index = np.asarray(edge_index)
    edge_attr = np.asarray(edge_attr, dtype=np.float32)
    W1 = np.asarray(W1, dtype=np.float32)
    W2 = np.ascontiguousarray(np.asarray(W2, dtype=np.float32))
    Wu = np.asarray(Wu, dtype=np.float32)
    b1 = np.asarray(b1, np.float32)
    bu = np.asarray(bu, np.float32)

    if cfg is None:
        cfg = Cfg(N_NODES, N_EDGES, N_CORES)
    cfg.triv = dict(
        b1=_is_const(b1, 0), b2=_is_const(b2, 0), bu=_is_const(bu, 0),
        g1=_is_const(g1, 1), be1=_is_const(be1, 0),
        gu=_is_const(gu, 1), beu=_is_const(beu, 0),
    )

    core_arrays = preprocess(cfg, x, edge_index, edge_attr)

    # Center the pre-LN linear maps so matmul outputs are mean-free.
    W1c = _center(W1)
    b1c = b1 - b1.mean()
    Wuc = _center(Wu)
    buc = bu - bu.mean()

    weights = dict(
        W1a=np.ascontiguousarray(W1c[0:D]).astype(bf16),
        W1b=np.ascontiguousarray(W1c[D:2 * D]).astype(bf16),
        W2=W2.astype(bf16),
        Wu_x=np.ascontiguousarray(Wuc[0:D]).astype(bf16),
        Wu_a=np.ascontiguousarray(Wuc[D:2 * D]).astype(bf16),
        c0=np.ascontiguousarray(W1c[2 * D]).astype(bf16),
        c1=np.ascontiguousarray(W1c[2 * D + 1]).astype(bf16),
        b1=b1c.astype(bf16),
        b2=np.asarray(b2, np.float32).astype(bf16),
        g1=np.asarray(g1, np.float32).astype(bf16),
        be1=np.asarray(be1, np.float32).astype(bf16),
        bu=np.ascontiguousarray(buc),
        gu=np.ascontiguousarray(np.asarray(gu, np.float32)),
        beu=np.ascontiguousarray(np.asarray(beu, np.float32)),
    )

    nc = make_program(cfg)

    in_maps = []
    for k in range(cfg.n_cores):
        xl = np.zeros((cfg.nloc_pad, D), dtype=np.float32)
        xl[:cfg.nloc] = x[k * cfg.nloc:(k + 1) * cfg.nloc]
        m = dict(core_arrays[k])
        m["x_local"] = xl
        m["xT_local"] = np.ascontiguousarray(xl.T).astype(bf16)
        m.update(weights)
        in_maps.append(m)

    if not run:
        return nc, in_maps, cfg

    from concourse import bass_utils
    res = bass_utils.run_bass_kernel_spmd(
        nc, in_maps, core_ids=list(range(cfg.n_cores)),
        trace=bool(int(os.environ.get("KERNEL_TRACE", "0"))),
    )
    kernel.last_results = res
    outs = [r["out"][:cfg.nloc] for r in res.results]
    return np.concatenate(outs, axis=0)


kernel.last_results = None
